# revision 1
# baseline (speedup 1.0000x reference)
"""CapsuleLayer kernel for Trainium2 (8 NeuronCores, data-parallel over batch).

Math: the reference's dynamic-routing loop is degenerate — `delta` is summed
over the capsule axis and broadcast back, so the logits stay constant across
capsules and softmax stays uniform (1/16) for all 3 iterations. The module
therefore reduces exactly to

    t   = (conv2d(x, sum_c W[c]) + sum_c b[c]) / 16      # 16-out-channel conv
    out = t * |t| / (1 + t*t)                            # scalar squash

The capsule sum is folded into the conv weights on the host (conv is linear in
the weights), leaving a [O=16, I=64, 3, 3] VALID conv + pointwise epilogue.

Device strategy per core (8 images per core):
  - x for an image pair lives in SBUF as [128, 66, 66] (partitions = parity*64
    + in_channel), loaded with one contiguous 2.2 MB DMA.
  - The conv runs on the TensorEngine as 9 accumulating matmuls (one per
    3x3 tap, shifts expressed in the rhs access pattern), packed 8-wide into
    the 128x128 array with tile_position (2 row groups x 4 col groups =
    K=64, M=32 tiles).  PSUM tile [128, 2048] = 4 banks holds one image
    pair: bank = parity*2 + h_group, partitions 32j+o = 4 h-tiles x 16 ch.
  - Epilogue: u=Square(t+bias) and s=Sign(t+bias) on ScalarE, w=u+1,
    r~=1/w (reciprocal_approx_fast), v=1-r on ScalarE, f=v*s on VectorE.
  - f's valid 16-partition slices DMA straight into the [8, 65536] output
    with a strided access pattern.
"""

import numpy as np

N_CORES = 8
B_PER_CORE = 8  # 64 images / 8 cores


def _build_nc(
    repeat=1,
    loop_repeat=1,
    conv_bf16=False,
    parts=None,
    w1_gpsimd=False,
    x_bufs=2,
    out_act_ring=False,
    wave_split=False,
):
    # parts: subset of {"in", "mm", "epi", "out"} for bench attribution;
    # None = all. "cal" alone = loop-overhead calibration body.
    if parts is None:
        parts = {"in", "mm", "epi", "out"}
    import contextlib

    import concourse.bacc as bacc
    import concourse.mybir as mybir
    import concourse.tile as tile

    f32 = mybir.dt.float32
    cdt = mybir.dt.bfloat16 if conv_bf16 else f32
    # Bacc (not raw Bass): its finalize() runs move_matmul_waits_to_ldweights
    # + generate_event_semaphores, required for TRN2's 1-wait-per-instruction
    # limit (our first matmuls collect several Tile sem waits).
    nc = bacc.Bacc(None, target_bir_lowering=False, debug=False)

    x_d = nc.dram_tensor("x", [512, 66, 66], cdt, kind="ExternalInput")
    w_d = nc.dram_tensor("w", [128, 288], cdt, kind="ExternalInput")
    bv_d = nc.dram_tensor("bvec", [128, 1], f32, kind="ExternalInput")
    # Raw per-pair dump [pair, partition, bank*512]; unshuffled on the host.
    out_d = nc.dram_tensor("out", [4, 128, 2048], cdt, kind="ExternalOutput")

    with tile.TileContext(nc) as tc:
        with (
            tc.tile_pool(name="const", bufs=1) as cp,
            tc.tile_pool(name="xp", bufs=x_bufs) as xp,
            tc.tile_pool(name="psp", bufs=2, space="PSUM") as psp,
            tc.tile_pool(name="wk", bufs=2) as wk,
        ):
            w_t = cp.tile([128, 288], cdt)
            nc.sync.dma_start(out=w_t[:, :], in_=w_d[:, :])
            b_t = cp.tile([128, 1], f32)
            nc.sync.dma_start(out=b_t[:, :], in_=bv_d[:, :])

            if loop_repeat > 1:  # bench only: HW loop repeating the body
                loop_cm = tc.For_i(
                    0,
                    loop_repeat,
                    1,
                    hint_engines=(
                        mybir.EngineType.PE,
                        mybir.EngineType.Activation,
                        mybir.EngineType.DVE,
                        mybir.EngineType.SP,
                    ),
                )
            else:
                loop_cm = contextlib.nullcontext()
            with loop_cm:
              if parts == {"cal"}:
                cal_t = wk.tile([128, 16], f32, tag="cal")
                nc.vector.memset(cal_t[:, :], 0.0)
              for p4 in range(0 if parts == {"cal"} else 4 * repeat):
                p = p4 % 4
                x_t = xp.tile([128, 66, 66], cdt, tag="x")
                if "in" in parts:
                    nc.sync.dma_start(
                        out=x_t[:, :, :], in_=x_d[128 * p : 128 * (p + 1), :, :]
                    )
                ps = psp.tile([128, 2048], f32, tag="ps")

                def emit_mms(hg_list, bank_fn, x_t=x_t, ps=ps):
                    for t in range(9):
                        kh, kw = divmod(t, 3)
                        for hg in hg_list:
                            for rg in range(2):
                                k = bank_fn(rg, hg)
                                for j in range(4):
                                    h0 = (hg * 4 + j) * 8
                                    nc.tensor.matmul(
                                        ps[
                                            32 * j : 32 * j + 32,
                                            512 * k : 512 * k + 512,
                                        ],
                                        w_t[
                                            64 * rg : 64 * rg + 64,
                                            32 * t : 32 * t + 32,
                                        ],
                                        x_t[
                                            64 * rg : 64 * rg + 64,
                                            h0 + kh : h0 + kh + 8,
                                            kw : kw + 64,
                                        ],
                                        start=(t == 0),
                                        stop=(t == 8),
                                        tile_position=(64 * rg, 32 * j),
                                        skip_group_check=True,
                                    )

                def emit_epi(c0, cw, p, ps=ps):
                    # t = ps + bias ; u = t^2 ; s = sign(t)
                    pss = ps[:, c0 : c0 + cw]
                    u = wk.tile([128, cw], f32, tag="u")
                    s = wk.tile([128, cw], cdt, tag="s")
                    w1 = wk.tile([128, cw], f32, tag="w1")
                    r = wk.tile([128, cw], f32, tag="r")
                    v = wk.tile([128, cw], cdt, tag="v")
                    f = wk.tile([128, cw], cdt, tag="f")
                    nc.scalar.activation(
                        u[:, :], pss, mybir.ActivationFunctionType.Square,
                        bias=b_t[:, 0:1],
                    )
                    nc.scalar.activation(
                        s[:, :], pss, mybir.ActivationFunctionType.Sign,
                        bias=b_t[:, 0:1],
                    )
                    nc.vector.tensor_scalar_add(w1[:, :], u[:, :], 1.0)
                    nc.vector.reciprocal_approx_fast(r[:, :], w1[:, :])
                    # v = 1 - r = t^2/(1+t^2)
                    nc.scalar.activation(
                        v[:, :], r[:, :], mybir.ActivationFunctionType.Copy,
                        bias=1.0, scale=-1.0,
                    )
                    nc.vector.tensor_mul(f[:, :], v[:, :], s[:, :])
                    if "out" in parts:
                        nc.sync.dma_start(
                            out=out_d[p, :, c0 : c0 + cw], in_=f[:, :]
                        )

                if wave_split:
                    for hg in range(2):
                        if "mm" in parts:
                            emit_mms([hg], lambda rg, hg: hg * 2 + rg)
                        if "epi" in parts:
                            emit_epi(1024 * hg, 1024, p)
                else:
                    if "mm" in parts:
                        emit_mms([0, 1], lambda rg, hg: rg * 2 + hg)
                    if "epi" in parts:
                        emit_epi(0, 2048, p)
    # Run the Bacc pass pipeline (wait splitting, reg alloc, ...) now; the
    # axon/pjrt execute path binds the primitive without finalizing.
    nc.finalize()
    return nc


def _np_bf16(a):
    import ml_dtypes

    return np.ascontiguousarray(a.astype(ml_dtypes.bfloat16))


def _prep_weights(W, b):
    """[16,16,64,3,3] capsule weights -> [128, 288] lhsT blocks (pre-summed
    over capsules, /16 for the uniform routing probs, duplicated into both
    partition halves).  Bias -> [128, 1] per-partition vector."""
    Wsum = np.asarray(W, dtype=np.float32).sum(axis=0) / 16.0  # [16, 64, 3, 3]
    w_arr = np.zeros((128, 288), np.float32)
    for t in range(9):
        kh, kw = divmod(t, 3)
        blk = np.ascontiguousarray(Wsum[:, :, kh, kw].T)  # [64 in, 16 out]
        w_arr[0:64, 32 * t : 32 * t + 16] = blk
        w_arr[64:128, 32 * t : 32 * t + 16] = blk
    bsum = np.asarray(b, dtype=np.float32).sum(axis=0) / 16.0  # [16]
    bvec = np.zeros((128, 1), np.float32)
    for j in range(4):
        bvec[32 * j : 32 * j + 16, 0] = bsum
    return w_arr, bvec


def make_in_maps(x, W, b, conv_bf16=False):
    x = np.ascontiguousarray(np.asarray(x, dtype=np.float32))
    w_arr, bvec = _prep_weights(W, b)
    if conv_bf16:
        x = _np_bf16(x)
        w_arr = _np_bf16(w_arr)
    return [
        {
            "x": np.ascontiguousarray(
                x[c * B_PER_CORE : (c + 1) * B_PER_CORE].reshape(512, 66, 66)
            ),
            "w": w_arr,
            "bvec": bvec,
        }
        for c in range(N_CORES)
    ]


def gather_out(per_core_outs, wave=False):
    """Unshuffle raw [4, 128, 2048] per-core dumps into [64, 65536, 1] f32.

    partition = 32*j + oo (oo<16 valid);
    free = 512*bank + n, bank = 2*rg + hg (or 2*hg + rg when wave=True);
    out[b=2p+rg, oo*4096 + (hg*4+j)*512 + n]."""
    full = np.empty((64, 65536), np.float32)
    perm = (0, 4, 2, 3, 1, 5) if wave else (0, 3, 2, 4, 1, 5)
    for c, raw in enumerate(per_core_outs):
        r = np.asarray(raw, dtype=np.float32).reshape(4, 4, 32, 2, 2, 512)
        v = r[:, :, :16].transpose(*perm)  # -> [p, rg, oo, hg, j, n]
        full[c * 8 : (c + 1) * 8] = v.reshape(8, 65536)
    return full.reshape(64, 65536, 1)


def kernel(x, W, b):
    from concourse.bass_utils import run_bass_kernel_spmd

    nc = _build_nc(conv_bf16=True)
    in_maps = make_in_maps(x, W, b, conv_bf16=True)
    res = run_bass_kernel_spmd(nc, in_maps, list(range(N_CORES)))
    return gather_out([res.results[c]["out"] for c in range(N_CORES)])



# revision 4
# speedup vs baseline: 1.0612x; 1.0612x over previous
"""CapsuleLayer kernel for Trainium2 (8 NeuronCores, data-parallel over batch).

Math: the reference's dynamic-routing loop is degenerate — `delta` is summed
over the capsule axis and broadcast back, so the logits stay constant across
capsules and softmax stays uniform (1/16) for all 3 iterations. The module
therefore reduces exactly to

    t   = (conv2d(x, sum_c W[c]) + sum_c b[c]) / 16      # 16-out-channel conv
    out = t * |t| / (1 + t*t)                            # scalar squash

The capsule sum is folded into the conv weights on the host (conv is linear in
the weights), leaving a [O=16, I=64, 3, 3] VALID conv + pointwise epilogue.

Device strategy per core (8 images per core):
  - x for an image pair lives in SBUF as [128, 66, 66] (partitions = parity*64
    + in_channel), loaded with one contiguous 2.2 MB DMA.
  - The conv runs on the TensorEngine as 9 accumulating matmuls (one per
    3x3 tap, shifts expressed in the rhs access pattern), packed 8-wide into
    the 128x128 array with tile_position (2 row groups x 4 col groups,
    K=64, M=16 tiles).  PSUM tile [128, 2048] = 4 banks holds one image
    pair: bank = parity*2 + h_group, partitions 32j+o (o<16) = 4 h-tiles x
    16 ch (partitions 32j+16..32j+31 are never written).
  - Epilogue (3 engine passes total, ~5.3us/pair of engine time spread over
    ACT+DVE which run in the PE shadow):
      ACT : tb = Identity(ps + bias)            -> bf16  (only PSUM read)
      DVE : v  = SQUASH_V_ANT(tb)               -> bf16  (custom 8-slice op:
            w = tb^2+1; seed = bitwise_not(w)*c0; y1 = one-Newton 1/w;
            v = 1 - y1  ==  t^2/(1+t^2), max abs err ~1.7e-3)
      DVE : f  = (tb & 0x8000) | v  (scalar_tensor_tensor on int16 views —
            splices sign(t) onto v; v >= -2e-3 so the OR is exact)
  - Only the 64 valid partitions are written out: 4 DMAs of [16, 2048] per
    pair into out[4, 64, 2048] (halves output HBM traffic vs [128, ...]).
"""

import numpy as np

N_CORES = 8
B_PER_CORE = 8  # 64 images / 8 cores

# Chebyshev-minimax pair for the bitwise-not reciprocal seed (same constants
# as concourse's RECIPROCAL_APPROX_FAST).
_CHEB_C0 = -0.23549792
_CHEB_C1 = 2.0017324

_SQUASH_NAME = "SQUASH_V_ANT"


def _get_squash_op():
    """Author + register the fused squash-magnitude DVE op (idempotent).

    body: v = 1 - y1,  y1 = y0*(c1 - w*y0),  y0 = bitwise_not(w)*c0,
          w = Src0^2 + 1      — exactly 8 ALU stages.
    """
    import concourse.dve_ops as dve_ops

    for o in dve_ops.OPS:
        if o.name == _SQUASH_NAME:
            return o
    from concourse.dve_spec import AluOp, Bin, C0, C1, One, Spec, Src0, lower, sq
    from concourse.dve_uop import DveOpSpec

    w = sq(Src0) + One
    n = Bin(AluOp.BITWISE_NOT, w, w)
    y0 = n * C0
    y1 = y0 * (C1 - w * y0)
    body = One - y1

    def _ref(in0, in1, c0, c1, c2):
        x = np.asarray(in0).astype(np.float32)
        w = x * x + np.float32(1.0)
        nn = (~w.view(np.int32)).view(np.float32)
        y0 = nn * np.float32(c0)
        y1 = y0 * (np.float32(c1) - w * y0)
        return np.float32(1.0) - y1

    spec = Spec(body=body, reference=_ref)
    row = dve_ops._CUSTOM_DVE_ROW_BASE + len(dve_ops.OPS)
    dve_ops._SUB_OPCODE_FOR_NAME[_SQUASH_NAME] = row
    shas = {
        ver: DveOpSpec(
            name=_SQUASH_NAME, opcode=row, uops=lower(spec, ver=ver), rd1_en=False
        ).sha(ver)
        for ver in ("v3", "v4")
    }
    op = dve_ops.DveOp(_SQUASH_NAME, spec, subdim=False, uops_sha=shas)
    dve_ops.OPS.append(op)
    dve_ops.CUSTOM_DVE_SPECS[_SQUASH_NAME] = spec
    return op


def _build_nc(
    repeat=1,
    loop_repeat=1,
    conv_bf16=True,
    parts=None,
    x_bufs=2,
):
    # parts: subset of {"in", "mm", "epi", "out"} for bench attribution;
    # None = all. "cal" alone = loop-overhead calibration body.
    if parts is None:
        parts = {"in", "mm", "epi", "out"}
    import contextlib

    import concourse.bacc as bacc
    import concourse.mybir as mybir
    import concourse.tile as tile

    squash_op = _get_squash_op()

    f32 = mybir.dt.float32
    i16 = mybir.dt.int16
    cdt = mybir.dt.bfloat16 if conv_bf16 else f32
    # Bacc (not raw Bass): its finalize() runs move_matmul_waits_to_ldweights
    # + generate_event_semaphores, required for TRN2's 1-wait-per-instruction
    # limit (our first matmuls collect several Tile sem waits).
    nc = bacc.Bacc(None, target_bir_lowering=False, debug=False)

    x_d = nc.dram_tensor("x", [512, 66, 66], cdt, kind="ExternalInput")
    w_d = nc.dram_tensor("w", [128, 288], cdt, kind="ExternalInput")
    bv_d = nc.dram_tensor("bvec", [128, 1], f32, kind="ExternalInput")
    # Valid-partition dump [pair, 16j+o, bank*512]; unshuffled on the host.
    out_d = nc.dram_tensor("out", [4, 64, 2048], cdt, kind="ExternalOutput")

    with tile.TileContext(nc) as tc:
        with (
            tc.tile_pool(name="const", bufs=1) as cp,
            tc.tile_pool(name="xp", bufs=x_bufs) as xp,
            tc.tile_pool(name="psp", bufs=2, space="PSUM") as psp,
            tc.tile_pool(name="wk", bufs=2) as wk,
        ):
            w_t = cp.tile([128, 288], cdt)
            nc.sync.dma_start(out=w_t[:, :], in_=w_d[:, :])
            b_t = cp.tile([128, 1], f32)
            nc.sync.dma_start(out=b_t[:, :], in_=bv_d[:, :])
            # 0x8000 per-partition mask for the sign splice (int16 imm values
            # are not supported by the STT verifier, so use a const AP).
            mask_t = cp.tile([128, 1], i16)
            nc.vector.memset(mask_t[:, :], -32768)

            if loop_repeat > 1:  # bench only: HW loop repeating the body
                loop_cm = tc.For_i(
                    0,
                    loop_repeat,
                    1,
                    hint_engines=(
                        mybir.EngineType.PE,
                        mybir.EngineType.Activation,
                        mybir.EngineType.DVE,
                        mybir.EngineType.SP,
                    ),
                )
            else:
                loop_cm = contextlib.nullcontext()
            with loop_cm:
              if parts == {"cal"}:
                cal_t = wk.tile([128, 16], f32, tag="cal")
                nc.vector.memset(cal_t[:, :], 0.0)
              for p4 in range(0 if parts == {"cal"} else 4 * repeat):
                p = p4 % 4
                x_t = xp.tile([128, 66, 66], cdt, tag="x")
                if "in" in parts:
                    nc.sync.dma_start(
                        out=x_t[:, :, :], in_=x_d[128 * p : 128 * (p + 1), :, :]
                    )
                ps = psp.tile([128, 2048], f32, tag="ps")

                if "mm" in parts:
                    for t in range(9):
                        kh, kw = divmod(t, 3)
                        for hg in range(2):
                            for rg in range(2):
                                k = rg * 2 + hg
                                for j in range(4):
                                    h0 = (hg * 4 + j) * 8
                                    nc.tensor.matmul(
                                        ps[
                                            32 * j : 32 * j + 16,
                                            512 * k : 512 * k + 512,
                                        ],
                                        w_t[
                                            64 * rg : 64 * rg + 64,
                                            32 * t : 32 * t + 16,
                                        ],
                                        x_t[
                                            64 * rg : 64 * rg + 64,
                                            h0 + kh : h0 + kh + 8,
                                            kw : kw + 64,
                                        ],
                                        start=(t == 0),
                                        stop=(t == 8),
                                        tile_position=(64 * rg, 32 * j),
                                        skip_group_check=True,
                                    )

                if "epi" in parts:
                    tb = wk.tile([128, 2048], cdt, tag="tb")
                    v = wk.tile([128, 2048], cdt, tag="v")
                    f = wk.tile([128, 2048], cdt, tag="f")
                    # tb = t = ps + bias (only PSUM read; bias for generality)
                    nc.scalar.activation(
                        tb[:, :],
                        ps[:, :],
                        mybir.ActivationFunctionType.Identity,
                        bias=b_t[:, 0:1],
                    )
                    # v = tb^2/(1+tb^2), fused seed+1NR reciprocal
                    nc.vector._custom_dve(
                        squash_op,
                        out=v[:, :],
                        in0=tb[:, :],
                        s0=_CHEB_C0,
                        s1=_CHEB_C1,
                    )
                    # f = (tb & 0x8000) | v  — splice sign(t) onto v
                    nc.vector.scalar_tensor_tensor(
                        f[:, :].bitcast(i16),
                        tb[:, :].bitcast(i16),
                        mask_t[:, 0:1],
                        v[:, :].bitcast(i16),
                        mybir.AluOpType.bitwise_and,
                        mybir.AluOpType.bitwise_or,
                    )
                    if "out" in parts:
                        for j in range(4):
                            nc.sync.dma_start(
                                out=out_d[p, 16 * j : 16 * j + 16, :],
                                in_=f[32 * j : 32 * j + 16, :],
                            )
    # Run the Bacc pass pipeline (wait splitting, reg alloc, ...) now; the
    # axon/pjrt execute path binds the primitive without finalizing.
    nc.finalize()
    return nc


def _np_bf16(a):
    import ml_dtypes

    return np.ascontiguousarray(a.astype(ml_dtypes.bfloat16))


def _prep_weights(W, b):
    """[16,16,64,3,3] capsule weights -> [128, 288] lhsT blocks (pre-summed
    over capsules, /16 for the uniform routing probs, duplicated into both
    partition halves).  Bias -> [128, 1] per-partition vector."""
    Wsum = np.asarray(W, dtype=np.float32).sum(axis=0) / 16.0  # [16, 64, 3, 3]
    w_arr = np.zeros((128, 288), np.float32)
    for t in range(9):
        kh, kw = divmod(t, 3)
        blk = np.ascontiguousarray(Wsum[:, :, kh, kw].T)  # [64 in, 16 out]
        w_arr[0:64, 32 * t : 32 * t + 16] = blk
        w_arr[64:128, 32 * t : 32 * t + 16] = blk
    bsum = np.asarray(b, dtype=np.float32).sum(axis=0) / 16.0  # [16]
    bvec = np.zeros((128, 1), np.float32)
    for j in range(4):
        bvec[32 * j : 32 * j + 16, 0] = bsum
    return w_arr, bvec


def make_in_maps(x, W, b, conv_bf16=True):
    x = np.ascontiguousarray(np.asarray(x, dtype=np.float32))
    w_arr, bvec = _prep_weights(W, b)
    if conv_bf16:
        x = _np_bf16(x)
        w_arr = _np_bf16(w_arr)
    return [
        {
            "x": np.ascontiguousarray(
                x[c * B_PER_CORE : (c + 1) * B_PER_CORE].reshape(512, 66, 66)
            ),
            "w": w_arr,
            "bvec": bvec,
        }
        for c in range(N_CORES)
    ]


def gather_out(per_core_outs):
    """Unshuffle raw [4, 64, 2048] per-core dumps into [64, 65536, 1] f32.

    partition row = 16*j + oo; free = 512*bank + n, bank = 2*rg + hg;
    out[b=2p+rg, oo*4096 + (hg*4+j)*512 + n]."""
    full = np.empty((64, 65536), np.float32)
    for c, raw in enumerate(per_core_outs):
        r = np.asarray(raw, dtype=np.float32).reshape(4, 4, 16, 2, 2, 512)
        v = r.transpose(0, 3, 2, 4, 1, 5)  # [p, rg, oo, hg, j, n]
        full[c * 8 : (c + 1) * 8] = v.reshape(8, 65536)
    return full.reshape(64, 65536, 1)


def kernel(x, W, b):
    from concourse.bass_utils import run_bass_kernel_spmd

    nc = _build_nc(conv_bf16=True)
    in_maps = make_in_maps(x, W, b, conv_bf16=True)
    res = run_bass_kernel_spmd(nc, in_maps, list(range(N_CORES)))
    return gather_out([res.results[c]["out"] for c in range(N_CORES)])
